# revision 12
# baseline (speedup 1.0000x reference)
"""MegablockMoE kernel for 8 Trainium2 NeuronCores.

Strategy (per sharding hint): expert-parallel. The router + token
dispatch/combine permutations (pure index bookkeeping) run on host as the
shard/unshard step; each of the 8 cores owns one expert and runs the two big
GEMMs (gelu(xg @ w1[e]) @ w2[e], 34.4 GFLOP/core) in bf16 with fp32 PSUM
accumulation, weights resident in SBUF, hT intermediate never leaving chip.

Device kernel (identical NEFF on all 8 cores, SPMD over experts):
    in : xgT [D, C] bf16   -- gathered tokens for this expert, transposed
         w1  [D, DFF] bf16, w2 [DFF, D] bf16
    mid: hT  [DFF, c_tile] bf16 = gelu(w1.T @ xgT)   (exact erf gelu, SBUF)
    out: yT  [D, C] bf16   = w2.T @ hT   (bf16: feeds only the host combine)

Measured on this backend: a pure bf16 matmul stream at N=256 moving
columns runs at ~130 ns/instruction regardless of weight reuse,
accumulation-group length, or eviction structure, so the kernel's 4096
matmuls are PE-streaming-bound. c_tile=256 beats 128 and 512 per-work;
fp8 DoubleRow (2x contraction/instr) fails the accuracy budget plain
(5.5e-2) and costs 12-vs-8 instructions compensated. mm1 and mm2 run
phase-separated per c-tile; mm2 keeps one 32-step accumulation group
per output chunk (seg_len=32), evicted by copies alternating DVE/ACT.
"""

import numpy as np
import ml_dtypes

import concourse.mybir as mybir
import concourse.tile as tile
from concourse import bacc
from concourse.bass_utils import run_bass_kernel_spmd

B, S, D = 4, 2048, 1024
E, K, DFF = 8, 2, 4096
T = B * S
C = K * T // E  # 2048 expert capacity
BF16 = ml_dtypes.bfloat16
N_CORES = 8

KO1, KO2 = D // 128, DFF // 128  # 8, 32
W1_CH = 8            # f-chunks of w1 (separate tiles -> fine-grained DMA deps)
W1_F = DFF // W1_CH  # 512
W2_CH = 8            # o-chunks of w2
W2_O = KO2 // W2_CH  # 4

# Hybrid-precision mm2: the last 2*N_DR8 of the 32 DFF chunks contract in
# fp8e4 DoubleRow pairs (half the instructions on that fraction), the rest
# in bf16. bf16 w2 chunks are pre-scaled by 2^12 (exact exponent shift) so
# both paths accumulate in one PSUM at the same scale; y eviction applies
# 2^-12. Measured rel err 3.8e-3 -> ~1.7e-2, still under the 2e-2 gate.
N_DR8 = 3                  # DR pairs
F_DR0 = KO2 - 2 * N_DR8    # first fp8 chunk (26)
W2_SCALE = float(2 ** 12)

_NC = None


def _build_nc(c_tile=256, n_iters=1, seg_len=32, structure="phased",
              debug=True):
    nc = bacc.Bacc(None, target_bir_lowering=False, debug=debug)
    xgT = nc.dram_tensor("xgT", [D, C], mybir.dt.bfloat16, kind="ExternalInput")
    w1 = nc.dram_tensor("w1", [D, DFF], mybir.dt.bfloat16, kind="ExternalInput")
    w2 = nc.dram_tensor("w2", [DFF, D], mybir.dt.bfloat16, kind="ExternalInput")
    w28 = nc.dram_tensor("w28", [128, N_DR8 * 2 * D], mybir.dt.float8e4,
                         kind="ExternalInput")
    # y leaves the device in bf16: it only feeds the host-side weighted
    # combine, and bf16 here costs 3.4e-3 -> 3.8e-3 rel err while halving
    # the DMA-out and result-fetch volume. Segmented mm2 (seg_len < 32)
    # accumulates partial sums in y, so it keeps fp32.
    y_dt = mybir.dt.bfloat16 if seg_len >= KO2 else mybir.dt.float32
    yT = nc.dram_tensor("yT", [D, C], y_dt, kind="ExternalOutput")

    xgT_v = xgT.rearrange("(o p) c -> p o c", p=128)
    w1_v = w1.rearrange("(o p) f -> p o f", p=128)
    w2_v = w2.rearrange("(o p) d -> p o d", p=128)
    yT_v = yT.rearrange("(o p) c -> p o c", p=128)
    n_ct = C // c_tile
    n_seg = KO2 // seg_len

    # SBUF budget (~208KB/partition): w1+w2 are 128KB resident; hT is
    # 16KB/partition per c_tile=256 buffer, 32KB at c_tile=512 — shrink the
    # hT/y pools at larger tiles. PE serialization makes hpool=1 stall-free.
    hp_bufs = 2 if c_tile <= 256 else 1
    yp_bufs = 2 if c_tile <= 256 else 1
    with tile.TileContext(nc) as tc:
        with (
            tc.tile_pool(name="wpool", bufs=1) as wpool,
            tc.tile_pool(name="xpool", bufs=3) as xpool,
            tc.tile_pool(name="hpool", bufs=hp_bufs) as hpool,
            tc.tile_pool(name="ypool", bufs=yp_bufs) as ypool,
            tc.tile_pool(name="ps1", bufs=3, space="PSUM") as ps1,
            tc.tile_pool(name="ps2", bufs=5, space="PSUM") as ps2,
        ):
            # first xg tile before weights: small and needed immediately
            xg_tiles = {}
            if n_iters == 1:
                xg_tiles[0] = xpool.tile([128, KO1, c_tile], mybir.dt.bfloat16,
                                         tag="xg", name="xg0")
                nc.sync.dma_start(xg_tiles[0][:], xgT_v[:, :, 0:c_tile])

            w1_tiles = []
            for ch in range(W1_CH):
                wt = wpool.tile([128, KO1, W1_F], mybir.dt.bfloat16,
                                tag=f"w1_{ch}", name=f"w1t{ch}")
                nc.sync.dma_start(wt[:], w1_v[:, :, ch * W1_F : (ch + 1) * W1_F])
                w1_tiles.append(wt)
            w28_sb = wpool.tile([128, N_DR8, 2, D], mybir.dt.float8e4,
                                tag="w28", name="w28t")
            nc.sync.dma_start(
                w28_sb[:],
                w28.rearrange("p (a b d) -> p a b d", a=N_DR8, b=2))
            w2_tiles = []
            for ch in range(W2_CH):
                wt = wpool.tile([128, W2_O, D], mybir.dt.bfloat16,
                                tag=f"w2_{ch}", name=f"w2t{ch}")
                nc.sync.dma_start(wt[:], w2_v[:, ch * W2_O : (ch + 1) * W2_O, :])
                w2_tiles.append(wt)

            def w1_ap(o, f):
                ch, r = divmod(f, W1_F // 128)
                return w1_tiles[ch][:, o, r * 128 : (r + 1) * 128]

            def w2_ap(f, g):
                ch, r = divmod(f, W2_O)
                return w2_tiles[ch][:, r, g * 128 : (g + 1) * 128]

            def evict_y(y_sb, g, psum):
                inv = 1.0 / W2_SCALE
                if g % 2 == 0:
                    nc.vector.tensor_scalar(y_sb[:, g, :], psum[:], inv, None,
                                            op0=mybir.AluOpType.mult)
                else:
                    nc.scalar.mul(y_sb[:, g, :], psum[:], inv)

            def body_phased(t, cs, xg_sb):
                hT_sb = hpool.tile([128, F_DR0, c_tile], mybir.dt.bfloat16,
                                   tag="hT")
                hT8_sb = hpool.tile([128, N_DR8, 2, c_tile],
                                    mybir.dt.float8e4, tag="hT8")
                y_sb = ypool.tile([128, KO1, c_tile], y_dt,
                                  tag="y")

                # mm1: 8-step accumulation groups, gelu eviction on ACT.
                # h chunks below F_DR0 evict to bf16, the rest to fp8e4
                # DoubleRow pair planes (values unscaled; h fits e4m3).
                for f in range(KO2):
                    psum = ps1.tile([128, c_tile], mybir.dt.float32,
                                    tag="p1")
                    for o in range(KO1):
                        nc.tensor.matmul(
                            psum[:], w1_ap(o, f), xg_sb[:, o, :],
                            start=(o == 0), stop=(o == KO1 - 1),
                        )
                    if f < F_DR0:
                        h_out = hT_sb[:, f, :]
                    else:
                        h_out = hT8_sb[:, (f - F_DR0) // 2, (f - F_DR0) % 2, :]
                    nc.scalar.activation(
                        h_out, psum[:],
                        mybir.ActivationFunctionType.Gelu,
                    )

                # mm2: one accumulation group per g: F_DR0 bf16 steps (w2
                # pre-scaled 2^12) + N_DR8 fp8 DoubleRow steps (w28 at the
                # same scale), evicted with a 2^-12 rescale
                for g in range(KO1):
                    psum = ps2.tile([128, c_tile], mybir.dt.float32,
                                    tag="p2")
                    for f in range(F_DR0):
                        nc.tensor.matmul(
                            psum[:], w2_ap(f, g), hT_sb[:, f, :],
                            start=(f == 0), stop=False,
                        )
                    for p in range(N_DR8):
                        nc.tensor.matmul(
                            psum[:],
                            w28_sb[:, p, :, g * 128:(g + 1) * 128],
                            hT8_sb[:, p, :, :],
                            start=False, stop=(p == N_DR8 - 1),
                            perf_mode=mybir.MatmulPerfMode.DoubleRow,
                        )
                    evict_y(y_sb, g, psum)
                nc.sync.dma_start(yT_v[:, :, cs], y_sb[:])

            def body_interleaved(t, cs, xg_sb):
                # original structure: mm1 f-loop with mm2 g0-3 interleaved
                # (accumulating across all f), then pass B for g4-7
                hT_sb = hpool.tile([128, KO2, c_tile], mybir.dt.bfloat16,
                                   tag="hT")
                y_sb = ypool.tile([128, KO1, c_tile], y_dt,
                                  tag="y")
                ps2g = [ps2.tile([128, c_tile], mybir.dt.float32,
                                 tag="p2", name=f"p2a{t}_{g}")
                        for g in range(4)]
                for f in range(KO2):
                    psum = ps1.tile([128, c_tile], mybir.dt.float32,
                                    tag="p1")
                    for o in range(KO1):
                        nc.tensor.matmul(
                            psum[:], w1_ap(o, f), xg_sb[:, o, :],
                            start=(o == 0), stop=(o == KO1 - 1),
                        )
                    nc.scalar.activation(
                        hT_sb[:, f, :], psum[:],
                        mybir.ActivationFunctionType.Gelu,
                    )
                    for g in range(4):
                        nc.tensor.matmul(
                            ps2g[g][:], w2_ap(f, g), hT_sb[:, f, :],
                            start=(f == 0), stop=(f == KO2 - 1),
                        )
                for g in range(4):
                    evict_y(y_sb, g, ps2g[g])
                ps2h = [ps2.tile([128, c_tile], mybir.dt.float32,
                                 tag="p2", name=f"p2b{t}_{g}")
                        for g in range(4)]
                for f in range(KO2):
                    for g in range(4):
                        nc.tensor.matmul(
                            ps2h[g][:], w2_ap(f, g + 4), hT_sb[:, f, :],
                            start=(f == 0), stop=(f == KO2 - 1),
                        )
                for g in range(4):
                    evict_y(y_sb, g + 4, ps2h[g])
                nc.sync.dma_start(yT_v[:, :, cs], y_sb[:])

            def body(_=None):
                for t in range(n_ct):
                    cs = slice(t * c_tile, (t + 1) * c_tile)
                    if t not in xg_tiles:
                        xg_tiles[t] = xpool.tile(
                            [128, KO1, c_tile], mybir.dt.bfloat16, tag="xg",
                            name=f"xg{t}",
                        )
                        nc.sync.dma_start(xg_tiles[t][:], xgT_v[:, :, cs])
                    if structure == "phased":
                        body_phased(t, cs, xg_tiles[t])
                    else:
                        body_interleaved(t, cs, xg_tiles[t])

            if n_iters == 1:
                body()
            else:
                with tc.For_i(0, n_iters, 1):
                    body()
    nc.compile()
    return nc


def _get_nc():
    global _NC
    if _NC is None:
        _NC = _build_nc()
    return _NC


def _route(x, wr):
    """Replicates the reference router exactly (fp32 numpy)."""
    xt = np.transpose(x, (1, 0, 2)).reshape(T, D)  # [T, D] fp32
    logits = xt.astype(np.float32) @ wr.astype(np.float32)  # [T, E]
    m = logits.max(axis=-1, keepdims=True)
    p = np.exp(logits - m, dtype=np.float32)
    p /= p.sum(axis=-1, keepdims=True)
    top1 = np.argmax(p, axis=-1)
    p_masked = p.copy()
    p_masked[np.arange(T), top1] = -np.inf
    top2 = np.argmax(p_masked, axis=-1)
    eidx = np.stack([top1, top2], axis=1)  # [T, K]
    ew = np.take_along_axis(p, eidx, axis=1).astype(np.float32)  # [T, K]

    flat_e = eidx.reshape(-1)
    order = np.argsort(flat_e, kind="stable")
    sorted_e = flat_e[order]
    hist = np.bincount(flat_e, minlength=E)
    starts = np.cumsum(hist) - hist
    pos = np.arange(T * K) - starts[sorted_e]
    keep = pos < C
    slot = np.where(keep, sorted_e * C + pos, E * C)
    token = order // K
    return xt, ew, order, keep, slot, token


def _make_in_maps(x, wr, w1, w2):
    xt, ew, order, keep, slot, token = _route(x, wr)
    slot_token = np.zeros(E * C, np.int64)
    slot_token[slot[keep]] = token[keep]
    filled = np.zeros(E * C, bool)
    filled[slot[keep]] = True
    xT_bf = np.ascontiguousarray(xt.T.astype(BF16))  # [D, T]
    E4 = ml_dtypes.float8_e4m3
    in_maps = []
    for e in range(E):
        idx = slot_token[e * C : (e + 1) * C]
        xgT_e = xT_bf[:, idx].copy()
        xgT_e[:, ~filled[e * C : (e + 1) * C]] = 0
        w2s = (w2[e] * W2_SCALE).astype(np.float32)  # exact exponent shift
        # fp8 pair-packed tail chunks of w2: w28[k, p, i, :] =
        # e4m3(w2[(F_DR0+2p+i)*128 + k, :] * 2^12)
        w28_e = np.empty((128, N_DR8, 2, D), E4)
        for p in range(N_DR8):
            for i in range(2):
                rows = slice((F_DR0 + 2 * p + i) * 128,
                             (F_DR0 + 2 * p + i + 1) * 128)
                w28_e[:, p, i, :] = w2s[rows, :].astype(E4)
        in_maps.append(
            {
                "xgT": np.ascontiguousarray(xgT_e),
                "w1": np.ascontiguousarray(w1[e].astype(BF16)),
                "w2": np.ascontiguousarray(w2s.astype(BF16)),
                "w28": np.ascontiguousarray(
                    w28_e.reshape(128, N_DR8 * 2 * D)),
            }
        )
    return in_maps, (ew, order, keep, slot)


def kernel(x, wr, w1, w2):
    nc = _get_nc()
    in_maps, (ew, order, keep, slot) = _make_in_maps(x, wr, w1, w2)

    res = run_bass_kernel_spmd(nc, in_maps, core_ids=list(range(N_CORES)))

    # --- combine: weighted scatter back to tokens ---
    Y = np.empty((E * C, D), np.float32)
    for e in range(E):
        Y[e * C : (e + 1) * C] = res.results[e]["yT"].T

    inv = np.empty(T * K, np.int64)
    inv[order] = np.arange(T * K)
    slot_tk = slot[inv].reshape(T, K)
    keep_tk = keep[inv].reshape(T, K)

    out_flat = np.zeros((T, D), np.float32)
    for k in range(K):
        sl = np.clip(slot_tk[:, k], 0, E * C - 1)
        contrib = Y[sl] * ew[:, k : k + 1]
        contrib[~keep_tk[:, k]] = 0.0
        out_flat += contrib
    return np.ascontiguousarray(
        out_flat.reshape(S, B, D).transpose(1, 0, 2)
    ).astype(np.float32)


# ---------------------------------------------------------------------------
# Benchmark helper (used by test.py; not part of the grading contract).
# ---------------------------------------------------------------------------


def make_bench(in_maps):
    import jax
    from jax.experimental.shard_map import shard_map
    from jax.sharding import Mesh, PartitionSpec, NamedSharding
    from concourse.bass2jax import (
        _bass_exec_p,
        install_neuronx_cc_hook,
        partition_id_tensor,
    )

    nc = _NC if _NC is not None else _get_nc()
    install_neuronx_cc_hook()
    partition_name = nc.partition_id_tensor.name if nc.partition_id_tensor else None

    in_names, out_names, out_avals, zero_outs = [], [], [], []
    for alloc in nc.m.functions[0].allocations:
        if not isinstance(alloc, mybir.MemoryLocationSet):
            continue
        name = alloc.memorylocations[0].name
        if alloc.kind == "ExternalInput":
            if name != partition_name:
                in_names.append(name)
        elif alloc.kind == "ExternalOutput":
            shape = tuple(alloc.tensor_shape)
            dtype = mybir.dt.np(alloc.dtype)
            out_avals.append(jax.core.ShapedArray(shape, dtype))
            zero_outs.append(np.zeros(shape, dtype))
            out_names.append(name)
    n_params = len(in_names)
    all_in_names = list(in_names) + list(out_names)
    if partition_name is not None:
        all_in_names.append(partition_name)
    if nc.dbg_addr is not None:
        dbg_zero = np.zeros((1, 2), np.uint32)
        in_maps = [{**m, nc.dbg_addr.name: dbg_zero} for m in in_maps]

    def _body(*args):
        operands = list(args)
        if partition_name is not None:
            operands.append(partition_id_tensor())
        outs = _bass_exec_p.bind(
            *operands,
            out_avals=tuple(out_avals),
            in_names=tuple(all_in_names),
            out_names=tuple(out_names),
            lowering_input_output_aliases=(),
            sim_require_finite=True,
            sim_require_nnan=True,
            nc=nc,
        )
        return tuple(outs)

    devices = jax.devices()[:N_CORES]
    mesh = Mesh(np.asarray(devices), ("core",))
    n_outs = len(out_names)
    in_specs = (PartitionSpec("core"),) * (n_params + n_outs)
    out_specs = (PartitionSpec("core"),) * n_outs
    fn = jax.jit(
        shard_map(_body, mesh=mesh, in_specs=in_specs, out_specs=out_specs,
                  check_rep=False),
        keep_unused=True,
    )
    concat_in = [
        np.concatenate([np.asarray(in_maps[c][name]) for c in range(N_CORES)],
                       axis=0)
        for name in in_names
    ]
    concat_zeros = [
        np.zeros((N_CORES * z.shape[0], *z.shape[1:]), z.dtype)
        for z in zero_outs
    ]
    shard = NamedSharding(mesh, PartitionSpec("core"))
    args = [jax.device_put(a, shard) for a in concat_in + concat_zeros]
    return fn, args, out_names


def benchmark(in_maps, iters=20, warmup=3):
    import time
    import jax

    fn, args, out_names = make_bench(in_maps)
    for _ in range(warmup):
        out = fn(*args)
        jax.block_until_ready(out)
    times = []
    for _ in range(iters):
        t0 = time.perf_counter()
        out = fn(*args)
        jax.block_until_ready(out)
        times.append(time.perf_counter() - t0)
    return min(times), sorted(times)[len(times) // 2], out


# revision 14
# speedup vs baseline: 1.1505x; 1.1505x over previous
"""MegablockMoE kernel for 8 Trainium2 NeuronCores.

Strategy (per sharding hint): expert-parallel. The router + token
dispatch/combine permutations (pure index bookkeeping) run on host as the
shard/unshard step; each of the 8 cores owns one expert and runs the two big
GEMMs (gelu(xg @ w1[e]) @ w2[e], 34.4 GFLOP/core) in bf16 with fp32 PSUM
accumulation, weights resident in SBUF, hT intermediate never leaving chip.

Device kernel (identical NEFF on all 8 cores, SPMD over experts):
    in : xgT [D, C] bf16   -- gathered tokens for this expert, transposed
         w1  [D, DFF] bf16, w2 [DFF, D] bf16
    mid: hT  [DFF, c_tile] bf16 = gelu(w1.T @ xgT)   (exact erf gelu, SBUF)
    out: yT  [D, C] bf16   = w2.T @ hT   (bf16: feeds only the host combine)

Measured on this backend: a matmul stream at N=256 moving columns runs
at ~130 ns/instruction regardless of dtype (bf16 == fp16 == fp8-DR),
weight reuse, accumulation-group length, or eviction structure — so the
kernel is PE-streaming-bound and the only lever is instruction count.
c_tile=256 beats 128 and 512 per-work. Full fp8 fails the accuracy gate
(5.5e-2 vs 2e-2) and 3-term compensation costs 12-vs-8 instructions, but
a PARTIAL fp8 contraction works: the last 6 of mm2's 32 DFF chunks run
as 3 fp8e4 DoubleRow pair instructions (2x contraction each), cutting
total instructions 4096 -> 3904 (-4.7%) at rel err 1.71e-2 (gate 2e-2,
deterministic inputs). mm1 and mm2 run phase-separated per c-tile; mm2
is one 32-step accumulation group per output chunk, bf16 steps first
(w2 pre-scaled 2^12 so both dtypes share one PSUM scale), evicted with
a 2^-12 rescale alternating DVE/ACT.
"""

import numpy as np
import ml_dtypes

import concourse.mybir as mybir
import concourse.tile as tile
from concourse import bacc
from concourse.bass_utils import run_bass_kernel_spmd

B, S, D = 4, 2048, 1024
E, K, DFF = 8, 2, 4096
T = B * S
C = K * T // E  # 2048 expert capacity
BF16 = ml_dtypes.bfloat16
N_CORES = 8

KO1, KO2 = D // 128, DFF // 128  # 8, 32
W1_CH = 8            # f-chunks of w1 (separate tiles -> fine-grained DMA deps)
W1_F = DFF // W1_CH  # 512
W2_CH = 8            # o-chunks of w2
W2_O = KO2 // W2_CH  # 4

# Hybrid-precision mm2: the last 2*N_DR8 of the 32 DFF chunks contract in
# fp8e4 DoubleRow pairs (half the instructions on that fraction), the rest
# in bf16. bf16 w2 chunks are pre-scaled by 2^12 (exact exponent shift) so
# both paths accumulate in one PSUM at the same scale; y eviction applies
# 2^-12. Measured rel err 3.8e-3 -> ~1.7e-2, still under the 2e-2 gate.
N_DR8 = 3                  # DR pairs
F_DR0 = KO2 - 2 * N_DR8    # first fp8 chunk (26)
W2_SCALE = float(2 ** 12)

_NC = None


def _build_nc(c_tile=256, n_iters=1, seg_len=32, structure="phased",
              debug=True):
    nc = bacc.Bacc(None, target_bir_lowering=False, debug=debug)
    xgT = nc.dram_tensor("xgT", [D, C], mybir.dt.bfloat16, kind="ExternalInput")
    w1 = nc.dram_tensor("w1", [D, DFF], mybir.dt.bfloat16, kind="ExternalInput")
    w2 = nc.dram_tensor("w2", [DFF, D], mybir.dt.bfloat16, kind="ExternalInput")
    w28 = nc.dram_tensor("w28", [128, N_DR8 * 2 * D], mybir.dt.float8e4,
                         kind="ExternalInput")
    # y leaves the device in bf16: it only feeds the host-side weighted
    # combine, and bf16 here costs 3.4e-3 -> 3.8e-3 rel err while halving
    # the DMA-out and result-fetch volume. Segmented mm2 (seg_len < 32)
    # accumulates partial sums in y, so it keeps fp32.
    y_dt = mybir.dt.bfloat16 if seg_len >= KO2 else mybir.dt.float32
    yT = nc.dram_tensor("yT", [D, C], y_dt, kind="ExternalOutput")

    xgT_v = xgT.rearrange("(o p) c -> p o c", p=128)
    w1_v = w1.rearrange("(o p) f -> p o f", p=128)
    w2_v = w2.rearrange("(o p) d -> p o d", p=128)
    yT_v = yT.rearrange("(o p) c -> p o c", p=128)
    n_ct = C // c_tile
    n_seg = KO2 // seg_len

    # SBUF budget (~208KB/partition): w1+w2 are 128KB resident; hT is
    # 16KB/partition per c_tile=256 buffer, 32KB at c_tile=512 — shrink the
    # hT/y pools at larger tiles. PE serialization makes hpool=1 stall-free.
    hp_bufs = 2 if c_tile <= 256 else 1
    yp_bufs = 2 if c_tile <= 256 else 1
    with tile.TileContext(nc) as tc:
        with (
            tc.tile_pool(name="wpool", bufs=1) as wpool,
            tc.tile_pool(name="xpool", bufs=3) as xpool,
            tc.tile_pool(name="hpool", bufs=hp_bufs) as hpool,
            tc.tile_pool(name="ypool", bufs=yp_bufs) as ypool,
            tc.tile_pool(name="ps1", bufs=3, space="PSUM") as ps1,
            tc.tile_pool(name="ps2", bufs=5, space="PSUM") as ps2,
        ):
            # first xg tile before weights: small and needed immediately
            xg_tiles = {}
            if n_iters == 1:
                xg_tiles[0] = xpool.tile([128, KO1, c_tile], mybir.dt.bfloat16,
                                         tag="xg", name="xg0")
                nc.sync.dma_start(xg_tiles[0][:], xgT_v[:, :, 0:c_tile])

            w1_tiles = []
            for ch in range(W1_CH):
                wt = wpool.tile([128, KO1, W1_F], mybir.dt.bfloat16,
                                tag=f"w1_{ch}", name=f"w1t{ch}")
                nc.sync.dma_start(wt[:], w1_v[:, :, ch * W1_F : (ch + 1) * W1_F])
                w1_tiles.append(wt)
            w28_sb = wpool.tile([128, N_DR8, 2, D], mybir.dt.float8e4,
                                tag="w28", name="w28t")
            nc.sync.dma_start(
                w28_sb[:],
                w28.rearrange("p (a b d) -> p a b d", a=N_DR8, b=2))
            w2_tiles = []
            for ch in range(W2_CH):
                wt = wpool.tile([128, W2_O, D], mybir.dt.bfloat16,
                                tag=f"w2_{ch}", name=f"w2t{ch}")
                nc.sync.dma_start(wt[:], w2_v[:, ch * W2_O : (ch + 1) * W2_O, :])
                w2_tiles.append(wt)

            def w1_ap(o, f):
                ch, r = divmod(f, W1_F // 128)
                return w1_tiles[ch][:, o, r * 128 : (r + 1) * 128]

            def w2_ap(f, g):
                ch, r = divmod(f, W2_O)
                return w2_tiles[ch][:, r, g * 128 : (g + 1) * 128]

            def evict_y(y_sb, g, psum):
                inv = 1.0 / W2_SCALE
                if g % 2 == 0:
                    nc.vector.tensor_scalar(y_sb[:, g, :], psum[:], inv, None,
                                            op0=mybir.AluOpType.mult)
                else:
                    nc.scalar.mul(y_sb[:, g, :], psum[:], inv)

            def body_phased(t, cs, xg_sb):
                hT_sb = hpool.tile([128, F_DR0, c_tile], mybir.dt.bfloat16,
                                   tag="hT")
                hT8_sb = hpool.tile([128, N_DR8, 2, c_tile],
                                    mybir.dt.float8e4, tag="hT8")
                y_sb = ypool.tile([128, KO1, c_tile], y_dt,
                                  tag="y")

                # mm1: 8-step accumulation groups, gelu eviction on ACT.
                # h chunks below F_DR0 evict to bf16, the rest to fp8e4
                # DoubleRow pair planes (values unscaled; h fits e4m3).
                for f in range(KO2):
                    psum = ps1.tile([128, c_tile], mybir.dt.float32,
                                    tag="p1")
                    for o in range(KO1):
                        nc.tensor.matmul(
                            psum[:], w1_ap(o, f), xg_sb[:, o, :],
                            start=(o == 0), stop=(o == KO1 - 1),
                        )
                    if f < F_DR0:
                        h_out = hT_sb[:, f, :]
                    else:
                        h_out = hT8_sb[:, (f - F_DR0) // 2, (f - F_DR0) % 2, :]
                    nc.scalar.activation(
                        h_out, psum[:],
                        mybir.ActivationFunctionType.Gelu,
                    )

                # mm2: one accumulation group per g: F_DR0 bf16 steps (w2
                # pre-scaled 2^12) + N_DR8 fp8 DoubleRow steps (w28 at the
                # same scale), evicted with a 2^-12 rescale
                for g in range(KO1):
                    psum = ps2.tile([128, c_tile], mybir.dt.float32,
                                    tag="p2")
                    for f in range(F_DR0):
                        nc.tensor.matmul(
                            psum[:], w2_ap(f, g), hT_sb[:, f, :],
                            start=(f == 0), stop=False,
                        )
                    for p in range(N_DR8):
                        nc.tensor.matmul(
                            psum[:],
                            w28_sb[:, p, :, g * 128:(g + 1) * 128],
                            hT8_sb[:, p, :, :],
                            start=False, stop=(p == N_DR8 - 1),
                            perf_mode=mybir.MatmulPerfMode.DoubleRow,
                        )
                    evict_y(y_sb, g, psum)
                nc.sync.dma_start(yT_v[:, :, cs], y_sb[:])

            def body_interleaved(t, cs, xg_sb):
                # original structure: mm1 f-loop with mm2 g0-3 interleaved
                # (accumulating across all f), then pass B for g4-7
                hT_sb = hpool.tile([128, KO2, c_tile], mybir.dt.bfloat16,
                                   tag="hT")
                y_sb = ypool.tile([128, KO1, c_tile], y_dt,
                                  tag="y")
                ps2g = [ps2.tile([128, c_tile], mybir.dt.float32,
                                 tag="p2", name=f"p2a{t}_{g}")
                        for g in range(4)]
                for f in range(KO2):
                    psum = ps1.tile([128, c_tile], mybir.dt.float32,
                                    tag="p1")
                    for o in range(KO1):
                        nc.tensor.matmul(
                            psum[:], w1_ap(o, f), xg_sb[:, o, :],
                            start=(o == 0), stop=(o == KO1 - 1),
                        )
                    nc.scalar.activation(
                        hT_sb[:, f, :], psum[:],
                        mybir.ActivationFunctionType.Gelu,
                    )
                    for g in range(4):
                        nc.tensor.matmul(
                            ps2g[g][:], w2_ap(f, g), hT_sb[:, f, :],
                            start=(f == 0), stop=(f == KO2 - 1),
                        )
                for g in range(4):
                    evict_y(y_sb, g, ps2g[g])
                ps2h = [ps2.tile([128, c_tile], mybir.dt.float32,
                                 tag="p2", name=f"p2b{t}_{g}")
                        for g in range(4)]
                for f in range(KO2):
                    for g in range(4):
                        nc.tensor.matmul(
                            ps2h[g][:], w2_ap(f, g + 4), hT_sb[:, f, :],
                            start=(f == 0), stop=(f == KO2 - 1),
                        )
                for g in range(4):
                    evict_y(y_sb, g + 4, ps2h[g])
                nc.sync.dma_start(yT_v[:, :, cs], y_sb[:])

            def body(_=None):
                for t in range(n_ct):
                    cs = slice(t * c_tile, (t + 1) * c_tile)
                    if t not in xg_tiles:
                        xg_tiles[t] = xpool.tile(
                            [128, KO1, c_tile], mybir.dt.bfloat16, tag="xg",
                            name=f"xg{t}",
                        )
                        nc.sync.dma_start(xg_tiles[t][:], xgT_v[:, :, cs])
                    if structure == "phased":
                        body_phased(t, cs, xg_tiles[t])
                    else:
                        body_interleaved(t, cs, xg_tiles[t])

            if n_iters == 1:
                body()
            else:
                with tc.For_i(0, n_iters, 1):
                    body()
    nc.compile()
    return nc


def _get_nc():
    global _NC
    if _NC is None:
        _NC = _build_nc()
    return _NC


def _route(x, wr):
    """Replicates the reference router exactly (fp32 numpy)."""
    xt = np.transpose(x, (1, 0, 2)).reshape(T, D)  # [T, D] fp32
    logits = xt.astype(np.float32) @ wr.astype(np.float32)  # [T, E]
    m = logits.max(axis=-1, keepdims=True)
    p = np.exp(logits - m, dtype=np.float32)
    p /= p.sum(axis=-1, keepdims=True)
    top1 = np.argmax(p, axis=-1)
    p_masked = p.copy()
    p_masked[np.arange(T), top1] = -np.inf
    top2 = np.argmax(p_masked, axis=-1)
    eidx = np.stack([top1, top2], axis=1)  # [T, K]
    ew = np.take_along_axis(p, eidx, axis=1).astype(np.float32)  # [T, K]

    flat_e = eidx.reshape(-1)
    order = np.argsort(flat_e, kind="stable")
    sorted_e = flat_e[order]
    hist = np.bincount(flat_e, minlength=E)
    starts = np.cumsum(hist) - hist
    pos = np.arange(T * K) - starts[sorted_e]
    keep = pos < C
    slot = np.where(keep, sorted_e * C + pos, E * C)
    token = order // K
    return xt, ew, order, keep, slot, token


def _make_in_maps(x, wr, w1, w2):
    xt, ew, order, keep, slot, token = _route(x, wr)
    slot_token = np.zeros(E * C, np.int64)
    slot_token[slot[keep]] = token[keep]
    filled = np.zeros(E * C, bool)
    filled[slot[keep]] = True
    xT_bf = np.ascontiguousarray(xt.T.astype(BF16))  # [D, T]
    E4 = ml_dtypes.float8_e4m3
    in_maps = []
    for e in range(E):
        idx = slot_token[e * C : (e + 1) * C]
        xgT_e = xT_bf[:, idx].copy()
        xgT_e[:, ~filled[e * C : (e + 1) * C]] = 0
        w2s = (w2[e] * W2_SCALE).astype(np.float32)  # exact exponent shift
        # fp8 pair-packed tail chunks of w2: w28[k, p, i, :] =
        # e4m3(w2[(F_DR0+2p+i)*128 + k, :] * 2^12)
        w28_e = np.empty((128, N_DR8, 2, D), E4)
        for p in range(N_DR8):
            for i in range(2):
                rows = slice((F_DR0 + 2 * p + i) * 128,
                             (F_DR0 + 2 * p + i + 1) * 128)
                w28_e[:, p, i, :] = w2s[rows, :].astype(E4)
        in_maps.append(
            {
                "xgT": np.ascontiguousarray(xgT_e),
                "w1": np.ascontiguousarray(w1[e].astype(BF16)),
                "w2": np.ascontiguousarray(w2s.astype(BF16)),
                "w28": np.ascontiguousarray(
                    w28_e.reshape(128, N_DR8 * 2 * D)),
            }
        )
    return in_maps, (ew, order, keep, slot)


def kernel(x, wr, w1, w2):
    # accept jax/device arrays too — the routing below is numpy
    x = np.asarray(x, dtype=np.float32)
    wr = np.asarray(wr, dtype=np.float32)
    w1 = np.asarray(w1, dtype=np.float32)
    w2 = np.asarray(w2, dtype=np.float32)
    nc = _get_nc()
    in_maps, (ew, order, keep, slot) = _make_in_maps(x, wr, w1, w2)

    res = run_bass_kernel_spmd(nc, in_maps, core_ids=list(range(N_CORES)))

    # --- combine: weighted scatter back to tokens ---
    Y = np.empty((E * C, D), np.float32)
    for e in range(E):
        Y[e * C : (e + 1) * C] = res.results[e]["yT"].T

    inv = np.empty(T * K, np.int64)
    inv[order] = np.arange(T * K)
    slot_tk = slot[inv].reshape(T, K)
    keep_tk = keep[inv].reshape(T, K)

    out_flat = np.zeros((T, D), np.float32)
    for k in range(K):
        sl = np.clip(slot_tk[:, k], 0, E * C - 1)
        contrib = Y[sl] * ew[:, k : k + 1]
        contrib[~keep_tk[:, k]] = 0.0
        out_flat += contrib
    return np.ascontiguousarray(
        out_flat.reshape(S, B, D).transpose(1, 0, 2)
    ).astype(np.float32)


# ---------------------------------------------------------------------------
# Benchmark helper (used by test.py; not part of the grading contract).
# ---------------------------------------------------------------------------


def make_bench(in_maps):
    import jax
    from jax.experimental.shard_map import shard_map
    from jax.sharding import Mesh, PartitionSpec, NamedSharding
    from concourse.bass2jax import (
        _bass_exec_p,
        install_neuronx_cc_hook,
        partition_id_tensor,
    )

    nc = _NC if _NC is not None else _get_nc()
    install_neuronx_cc_hook()
    partition_name = nc.partition_id_tensor.name if nc.partition_id_tensor else None

    in_names, out_names, out_avals, zero_outs = [], [], [], []
    for alloc in nc.m.functions[0].allocations:
        if not isinstance(alloc, mybir.MemoryLocationSet):
            continue
        name = alloc.memorylocations[0].name
        if alloc.kind == "ExternalInput":
            if name != partition_name:
                in_names.append(name)
        elif alloc.kind == "ExternalOutput":
            shape = tuple(alloc.tensor_shape)
            dtype = mybir.dt.np(alloc.dtype)
            out_avals.append(jax.core.ShapedArray(shape, dtype))
            zero_outs.append(np.zeros(shape, dtype))
            out_names.append(name)
    n_params = len(in_names)
    all_in_names = list(in_names) + list(out_names)
    if partition_name is not None:
        all_in_names.append(partition_name)
    if nc.dbg_addr is not None:
        dbg_zero = np.zeros((1, 2), np.uint32)
        in_maps = [{**m, nc.dbg_addr.name: dbg_zero} for m in in_maps]

    def _body(*args):
        operands = list(args)
        if partition_name is not None:
            operands.append(partition_id_tensor())
        outs = _bass_exec_p.bind(
            *operands,
            out_avals=tuple(out_avals),
            in_names=tuple(all_in_names),
            out_names=tuple(out_names),
            lowering_input_output_aliases=(),
            sim_require_finite=True,
            sim_require_nnan=True,
            nc=nc,
        )
        return tuple(outs)

    devices = jax.devices()[:N_CORES]
    mesh = Mesh(np.asarray(devices), ("core",))
    n_outs = len(out_names)
    in_specs = (PartitionSpec("core"),) * (n_params + n_outs)
    out_specs = (PartitionSpec("core"),) * n_outs
    fn = jax.jit(
        shard_map(_body, mesh=mesh, in_specs=in_specs, out_specs=out_specs,
                  check_rep=False),
        keep_unused=True,
    )
    concat_in = [
        np.concatenate([np.asarray(in_maps[c][name]) for c in range(N_CORES)],
                       axis=0)
        for name in in_names
    ]
    concat_zeros = [
        np.zeros((N_CORES * z.shape[0], *z.shape[1:]), z.dtype)
        for z in zero_outs
    ]
    shard = NamedSharding(mesh, PartitionSpec("core"))
    args = [jax.device_put(a, shard) for a in concat_in + concat_zeros]
    return fn, args, out_names


def benchmark(in_maps, iters=20, warmup=3):
    import time
    import jax

    fn, args, out_names = make_bench(in_maps)
    for _ in range(warmup):
        out = fn(*args)
        jax.block_until_ready(out)
    times = []
    for _ in range(iters):
        t0 = time.perf_counter()
        out = fn(*args)
        jax.block_until_ready(out)
        times.append(time.perf_counter() - t0)
    return min(times), sorted(times)[len(times) // 2], out
